# revision 23
# baseline (speedup 1.0000x reference)
"""AssignAttention (topk_masking) Trainium2 kernel — 8 NeuronCores.

Sharding: data-parallel over B (2 groups of 4 cores), tensor-parallel over
heads H (2 heads per core). Per core: QKV projections for its 2 heads (f32r
matmuls), rawT = k q^T per head, per-column top-4 via the DVE MAX8
instruction, binary Y^T mask, count matmuls, sparse-softmax reformulation
(exp(A) = 1 + (e_n-1) Y), Yv matmul, output projection with fused
normalization, ReduceScatter(add) over each 4-core group.

Math: with Y[n,s] = [n in top4 of column s], c_n = sum_s Y, cm_n = sum_s
mask_s Y, e_n = exp(1/(c_n+1)), M = sum_s mask_s, Z_n = M + (e_n-1) cm_n:
out_head[n,:] = (Vsum + (e_n-1) * (Y.mask @ v)[n,:]) / Z_n
which equals the reference's hard-topk + sum-normalize + masked softmax.
"""
import sys, os

os.environ["JAX_ENABLE_COMPILATION_CACHE"] = "false"
sys.path.insert(0, "/opt/trn_rl_repo")
import numpy as np
import ml_dtypes

B, N, C, H, K = 2, 2048, 1024, 8, 4
HD = C // H
SCALE = HD ** -0.5
NCORES = 8
ST = 16          # s-tiles per head
F32 = None       # set after import
BF16 = None

_cache = {}


def _build():
    from concourse import bacc, tile, mybir

    f32, f32r, bf16 = mybir.dt.float32, mybir.dt.float32r, mybir.dt.bfloat16
    AF = mybir.ActivationFunctionType
    OP = mybir.AluOpType

    nc = bacc.Bacc(None, target_bir_lowering=False)
    d_qt = nc.declare_dram_parameter("qt", [C, N], f32, isOutput=False)
    d_wq = nc.declare_dram_parameter("wq", [C, 2 * HD], f32, isOutput=False)
    d_wk = nc.declare_dram_parameter("wk", [C, 2 * HD], f32, isOutput=False)
    d_wv = nc.declare_dram_parameter("wv", [C, 2 * HD], f32, isOutput=False)
    d_wp = nc.declare_dram_parameter("wp", [2 * HD, C], f32, isOutput=False)
    d_maskT = nc.declare_dram_parameter("maskT", [128, ST], f32, isOutput=False)
    d_om = nc.declare_dram_parameter("om", [128, 2 * ST], bf16, isOutput=False)
    d_ones = nc.declare_dram_parameter("onesb", [128, 16], bf16, isOutput=False)
    d_mb = nc.declare_dram_parameter("mbcol", [128, 1], f32, isOutput=False)
    d_gate = nc.declare_dram_parameter("gatecol", [128, 1], f32, isOutput=False)
    d_idf = nc.declare_dram_parameter("idf", [128, 128], f32, isOutput=False)
    d_idb = nc.declare_dram_parameter("idb", [128, 128], bf16, isOutput=False)
    d_bq = nc.declare_dram_parameter("bqb", [128, 2], f32, isOutput=False)
    d_bk = nc.declare_dram_parameter("bkb", [128, 2], f32, isOutput=False)
    d_bv = nc.declare_dram_parameter("bvb", [128, 2], f32, isOutput=False)
    d_out = nc.declare_dram_parameter("out", [N // 4, C], f32, isOutput=True)
    d_dbg = nc.declare_dram_parameter("dbg", [128, 2112], f32, isOutput=True)

    from contextlib import ExitStack
    with tile.TileContext(nc) as tc:
        with (
            tc.tile_pool(name="cst", bufs=1) as cst,
            tc.tile_pool(name="qk", bufs=1) as qk,
            tc.tile_pool(name="ps_a", bufs=2, space="PSUM") as ps_a,
            tc.tile_pool(name="ps_cnt", bufs=1, space="PSUM") as ps_cnt,
            tc.tile_pool(name="ps_b", bufs=1, space="PSUM") as ps_b,
            tc.tile_pool(name="dram", bufs=1, space="DRAM") as dram,
        ):
            stage1 = ExitStack()
            qtp = stage1.enter_context(tc.tile_pool(name="qtp", bufs=1))
            tmp = stage1.enter_context(tc.tile_pool(name="tmp", bufs=3))
            # ---- constants in ----
            maskT = cst.tile([128, ST], f32)
            om = cst.tile([128, 2 * ST], bf16)
            onesb = cst.tile([128, 16], bf16)
            mbcol = cst.tile([128, 1], f32)
            gatecol = cst.tile([128, 1], f32)
            idf = cst.tile([128, 128], f32)
            idb = cst.tile([128, 128], bf16)
            bqb = cst.tile([128, 2], f32)
            bkb = cst.tile([128, 2], f32)
            bvb = cst.tile([128, 2], f32)
            for t, d in [(maskT, d_maskT), (om, d_om), (onesb, d_ones),
                         (mbcol, d_mb), (gatecol, d_gate), (idf, d_idf),
                         (idb, d_idb), (bqb, d_bq), (bkb, d_bk), (bvb, d_bv)]:
                nc.sync.dma_start(t[:], d[:])

            # ---- load + round queryT and weights to f32r ----
            qtr = qtp.tile([128, 8 * N], mybir.dt.float32r)   # 8 c-tiles stacked
            for i in range(8):
                t = tmp.tile([128, N], f32, tag="ld")
                nc.sync.dma_start(t[:], d_qt[i * 128:(i + 1) * 128, :])
                nc.vector.tensor_copy(qtr[:, i * N:(i + 1) * N], t[:])
            wtr = {}
            for nm, d_w in (("q", d_wq), ("k", d_wk), ("v", d_wv)):
                wr = qtp.tile([128, 8 * 2 * HD], mybir.dt.float32r, tag=f"w{nm}")
                for i in range(8):
                    t = tmp.tile([128, 2 * HD], f32, tag="ldw")
                    nc.sync.dma_start(t[:], d_w[i * 128:(i + 1) * 128, :])
                    nc.scalar.activation(wr[:, i * 2 * HD:(i + 1) * 2 * HD], t[:],
                                         AF.Copy, bias=0.0, scale=1.0)
                wtr[nm] = wr
            wpr = qtp.tile([128, 2 * C], mybir.dt.float32r)
            wpb = cst.tile([128, 2 * C], bf16)
            for h in range(2):
                t = tmp.tile([128, C], f32, tag="ld")
                nc.sync.dma_start(t[:], d_wp[h * 128:(h + 1) * 128, :])
                nc.scalar.activation(wpr[:, h * C:(h + 1) * C], t[:],
                                     AF.Copy, bias=0.0, scale=1.0)
                nc.vector.tensor_copy(wpb[:, h * C:(h + 1) * C], t[:])

            # ---- QKV projections (f32r), per head ----
            qT = [qk.tile([128, N], mybir.dt.float32r, tag=f"q{h}", name=f"qT{h}") for h in range(2)]
            kT = [qk.tile([128, N], mybir.dt.float32r, tag=f"k{h}", name=f"kT{h}") for h in range(2)]
            vTb = [qtp.tile([128, N], bf16, tag=f"v{h}", name=f"vTb{h}") for h in range(2)]
            for h in range(2):
                for nm, dst, bias_t, scale in (("q", qT[h], bqb, SCALE),
                                               ("k", kT[h], bkb, 1.0),
                                               ("v", vTb[h], bvb, 1.0)):
                    for ch in range(4):
                        ps = ps_a.tile([128, 512], f32, tag="a")
                        for ci in range(8):
                            lhs = wtr[nm][:, ci * 256 + h * 128: ci * 256 + (h + 1) * 128]
                            rhs = qtr[:, ci * N + ch * 512: ci * N + ch * 512 + 512]
                            nc.tensor.matmul(ps[:], lhs, rhs,
                                             start=(ci == 0), stop=(ci == 7))
                        nc.scalar.activation(dst[:, ch * 512:(ch + 1) * 512], ps[:],
                                             AF.Identity, bias=bias_t[:, h:h + 1],
                                             scale=scale)

            # ---- v transpose + mask; Vsum accumulation ----
            vm = [qk.tile([128, N], bf16, tag=f"vm{h}", name=f"vm{h}") for h in range(2)]
            vsum_r = []
            for h in range(2):
                for st in range(ST):
                    ps = ps_a.tile([128, 128], bf16, tag="a")
                    nc.tensor.transpose(ps[:], vTb[h][:, st * 128:(st + 1) * 128], idb[:])
                    nc.vector.tensor_scalar(vm[h][:, st * 128:(st + 1) * 128], ps[:],
                                            maskT[:, st:st + 1], None, OP.mult)
                pvs = ps_cnt.tile([128, 16], f32, tag="acc")
                for st in range(ST):
                    nc.tensor.matmul(pvs[:], vm[h][:, st * 128:(st + 1) * 128], onesb[:],
                                     start=(st == 0), stop=(st == ST - 1))
                vs = cst.tile([128, 1], mybir.dt.float32r, tag=f"vs{h}")
                nc.vector.tensor_copy(vs[:], pvs[:, 0:1])
                vsum_r.append(vs)
            # VsumP_h = Vsum_h @ Wp_h  -> [1, 1024] per head (r_n is per-head!)
            vsump = []
            for h in range(2):
                pvp = ps_b.tile([1, C], f32, tag="b")
                for ch in range(2):
                    nc.tensor.matmul(pvp[:, ch * 512:(ch + 1) * 512], vsum_r[h],
                                     wpr[:, h * C + ch * 512: h * C + ch * 512 + 512],
                                     start=True, stop=True)
                vp_h = cst.tile([1, C], bf16, tag=f"vsump{h}", name=f"vsump{h}")
                nc.vector.tensor_copy(vp_h[:], pvp[:])
                vsump.append(vp_h)
            vsumpf = vsump[0]  # debug alias

            stage1.close()
            stage2 = ExitStack()
            yb = stage2.enter_context(tc.tile_pool(name="yb", bufs=1))
            work = stage2.enter_context(tc.tile_pool(name="work", bufs=2))
            sres = stage2.enter_context(tc.tile_pool(name="sres", bufs=1))
            outp = stage2.enter_context(tc.tile_pool(name="outp", bufs=2))

            # ---- per head: topk, counts, Yv, normalize factors ----
            S_sb = [sres.tile([128, N], bf16, tag=f"s{h}", name=f"Ssb{h}") for h in range(2)]
            wcol = []   # w = r*em1 per head, [128, 16] fp32
            rgwT = []   # [16, 128] f32r per head
            for h in range(2):
                ybig = yb.tile([128, ST * N], bf16, tag="y")
                pyv = ps_cnt.tile([128, N], f32, tag="acc")
                for st in range(ST):
                    raw = work.tile([128, N], f32, tag="raw")
                    for ch in range(4):
                        ps = ps_a.tile([128, 512], f32, tag="a")
                        nc.tensor.matmul(ps[:], kT[h][:, st * 128:(st + 1) * 128],
                                         qT[h][:, ch * 512:(ch + 1) * 512],
                                         start=True, stop=True)
                        nc.scalar.activation(raw[:, ch * 512:(ch + 1) * 512], ps[:],
                                             AF.Copy, bias=0.0, scale=1.0)
                    top8 = work.tile([128, 8], f32, tag="top8")
                    nc.vector.max(top8[:], raw[:])
                    ytile = ybig[:, st * N:(st + 1) * N]
                    nc.vector.tensor_scalar(ytile, raw[:], top8[:, K - 1:K], None, OP.is_ge)
                    for ch in range(4):
                        nc.tensor.matmul(pyv[:, ch * 512:(ch + 1) * 512],
                                         vm[h][:, st * 128:(st + 1) * 128],
                                         ybig[:, st * N + ch * 512: st * N + ch * 512 + 512],
                                         start=(st == 0), stop=(st == ST - 1))
                for ch in range(4):
                    nc.scalar.activation(S_sb[h][:, ch * 512:(ch + 1) * 512],
                                         pyv[:, ch * 512:(ch + 1) * 512],
                                         AF.Copy, bias=0.0, scale=1.0)
                # counts matmuls (reuse the acc psum slot after S evac)
                pcnt = ps_cnt.tile([2, N], f32, tag="acc")
                for st in range(ST):
                    for ch in range(4):
                        nc.tensor.matmul(pcnt[:, ch * 512:(ch + 1) * 512],
                                         om[:, 2 * st:2 * st + 2],
                                         ybig[:, st * N + ch * 512: st * N + ch * 512 + 512],
                                         start=(st == 0), stop=(st == ST - 1))
                # counts -> [128, 32] via PE transpose
                cnt_sb = work.tile([2, N], f32, tag="cnt_sb", bufs=1)
                nc.vector.tensor_copy(cnt_sb[:], pcnt[:])
                ptr = ps_b.tile([128, 32], f32, tag="b")
                for t2 in range(ST):
                    nc.tensor.transpose(ptr[:, 2 * t2:2 * t2 + 2],
                                        cnt_sb[:, t2 * 128:(t2 + 1) * 128],
                                        idf[:2, :2])
                cntT = work.tile([128, 32], f32, tag="cntT")
                nc.vector.tensor_copy(cntT[:], ptr[:])
                cN = work.tile([128, 16], f32, tag="cN")
                cM = work.tile([128, 16], f32, tag="cM")
                nc.vector.tensor_copy(cN[:], cntT[:, 0:32:2])
                nc.vector.tensor_copy(cM[:], cntT[:, 1:32:2])
                rec = work.tile([128, 16], f32, tag="rec")
                nc.vector.tensor_scalar(rec[:], cN[:], 1.0, None, OP.add)
                nc.vector.reciprocal(rec[:], rec[:])
                e = work.tile([128, 16], f32, tag="e")
                nc.scalar.activation(e[:], rec[:], AF.Exp)
                em1 = work.tile([128, 16], f32, tag="em1")
                nc.vector.tensor_scalar(em1[:], e[:], -1.0, None, OP.add)
                Z = work.tile([128, 16], f32, tag="Z")
                nc.vector.tensor_mul(Z[:], em1[:], cM[:])
                nc.vector.tensor_scalar(Z[:], Z[:], mbcol[:, 0:1], None, OP.add)
                r_ = work.tile([128, 16], f32, tag="r_")
                nc.vector.reciprocal(r_[:], Z[:])
                w_ = work.tile([128, 16], f32, tag=f"w{h}_", name=f"w{h}_")
                nc.vector.tensor_mul(w_[:], r_[:], em1[:])
                wcol.append(w_)
                rem = work.tile([128, 16], f32, tag="rem")
                nc.vector.reciprocal(rem[:], em1[:])
                rgw = work.tile([128, 16], f32, tag="rgw")
                nc.vector.tensor_scalar(rgw[:], rem[:], gatecol[:, 0:1], None, OP.mult)
                prt = ps_b.tile([16, 128], f32, tag="b")
                nc.tensor.transpose(prt[:], rgw[:], idf[:])
                rgt16 = work.tile([16, 128], f32, tag="rgT16", bufs=1)
                nc.vector.tensor_copy(rgt16[:], prt[:])
                rgt1 = work.tile([1, 16 * 128], f32, tag="rgT1", bufs=1)
                nc.sync.dma_start(rgt1[:], rgt16[:])
                rgt = work.tile([1, 16 * 128], bf16, tag=f"rgTf{h}", name=f"rgTf{h}")
                nc.vector.tensor_copy(rgt[:], rgt1[:])
                rgwT.append(rgt)

            # w0 must be combined with w1: out = w0*S0@Wp0 + w1*S1@Wp1 + outer terms.
            # Since w differs per head, scale cannot be applied at a shared evac.
            # Instead: psum accumulates S0@Wp0*? -> need per-head scaling BEFORE the
            # matmul: scale S^T columns by w (free axis)... not possible. So:
            # evacuate per-head psums separately and add on DVE.
            partialA = dram.tile([N, 512], f32)
            partialB = dram.tile([N, 512], f32)
            rsA = dram.tile([N // 4, 512], f32)
            rsB = dram.tile([N // 4, 512], f32)
            for cch in range(2):
                partial_d = partialA if cch == 0 else partialB
                for nt in range(ST):
                    ob = outp.tile([128, 512], f32, tag="ob")
                    ps0 = ps_a.tile([128, 512], f32, tag="a")
                    nc.tensor.matmul(ps0[:], rgwT[0][0:1, nt * 128:(nt + 1) * 128],
                                     vsump[0][0:1, cch * 512:(cch + 1) * 512],
                                     start=True, stop=False)
                    nc.tensor.matmul(ps0[:], S_sb[0][:, nt * 128:(nt + 1) * 128],
                                     wpb[:, 0 * C + cch * 512: 0 * C + cch * 512 + 512],
                                     start=False, stop=True)
                    nc.scalar.activation(ob[:], ps0[:],
                                         AF.Copy, bias=0.0, scale=wcol[0][:, nt:nt + 1])
                    ps1 = ps_a.tile([128, 512], f32, tag="a")
                    nc.tensor.matmul(ps1[:], rgwT[1][0:1, nt * 128:(nt + 1) * 128],
                                     vsump[1][0:1, cch * 512:(cch + 1) * 512],
                                     start=True, stop=False)
                    nc.tensor.matmul(ps1[:], S_sb[1][:, nt * 128:(nt + 1) * 128],
                                     wpb[:, 1 * C + cch * 512: 1 * C + cch * 512 + 512],
                                     start=False, stop=True)
                    sc1 = outp.tile([128, 512], f32, tag="sc1")
                    nc.scalar.activation(sc1[:], ps1[:], AF.Copy, bias=0.0,
                                         scale=wcol[1][:, nt:nt + 1])
                    nc.vector.tensor_add(ob[:], ob[:], sc1[:])
                    nc.sync.dma_start(partial_d[nt * 128:(nt + 1) * 128, :], ob[:])
                rs_d = rsA if cch == 0 else rsB
                nc.gpsimd.collective_compute(
                    "ReduceScatter", OP.add,
                    replica_groups=[[0, 1, 2, 3], [4, 5, 6, 7]],
                    ins=[partial_d[:].opt()], outs=[rs_d[:].opt()])
                nc.gpsimd.dma_start(d_out[:, cch * 512:(cch + 1) * 512], rs_d[:])
            stage2.close()

    nc.compile()
    return nc


def _host_inputs(query, mask, Wq, bq, Wk, bk, Wv, bv, Wp, bp):
    """Per-core input dicts."""
    bf = ml_dtypes.bfloat16
    ins = []
    idf = np.eye(128, dtype=np.float32)
    idb = np.eye(128, dtype=bf)
    onesb = np.ones((128, 16), dtype=bf)
    for c in range(NCORES):
        b, g = c // 4, c % 4
        h0 = 2 * g
        qt = np.ascontiguousarray(query[b].T.astype(np.float32))
        sl = slice(h0 * HD, (h0 + 2) * HD)
        maskT = np.ascontiguousarray(
            mask[b].reshape(ST, 128).T.astype(np.float32))
        om = np.zeros((128, 2 * ST), dtype=bf)
        om[:, 0::2] = 1.0
        om[:, 1::2] = maskT.astype(bf)
        mbcol = np.full((128, 1), float(mask[b].sum()), dtype=np.float32)
        gatecol = np.full((128, 1), 1.0, dtype=np.float32)
        bqb = np.stack([SCALE * bq[(h0 + i) * HD:(h0 + i + 1) * HD] for i in range(2)],
                       axis=1).astype(np.float32)
        bkb = np.stack([bk[(h0 + i) * HD:(h0 + i + 1) * HD] for i in range(2)],
                       axis=1).astype(np.float32)
        bvb = np.stack([bv[(h0 + i) * HD:(h0 + i + 1) * HD] for i in range(2)],
                       axis=1).astype(np.float32)
        ins.append(dict(
            qt=qt,
            wq=np.ascontiguousarray(Wq[:, sl].astype(np.float32)),
            wk=np.ascontiguousarray(Wk[:, sl].astype(np.float32)),
            wv=np.ascontiguousarray(Wv[:, sl].astype(np.float32)),
            wp=np.ascontiguousarray(Wp[sl, :].astype(np.float32)),
            maskT=maskT, om=om, onesb=onesb, mbcol=mbcol, gatecol=gatecol,
            idf=idf, idb=idb, bqb=bqb, bkb=bkb, bvb=bvb))
    return ins


def kernel(query, mask, Wq, bq, Wk, bk, Wv, bv, Wp, bp):
    from concourse.bass_utils import run_bass_kernel_spmd

    if "nc" not in _cache:
        _cache["nc"] = _build()
    nc = _cache["nc"]
    ins = _host_inputs(query, mask, Wq, bq, Wk, bk, Wv, bv, Wp, bp)
    res = run_bass_kernel_spmd(nc, ins, list(range(NCORES)))
    out = np.empty((B, N, C), dtype=np.float32)
    for b in range(B):
        out[b] = np.concatenate(
            [res.results[4 * b + p]["out"] for p in range(4)], axis=0)
    out += np.asarray(bp, dtype=np.float32)[None, None, :]
    return out
